# revision 1
# baseline (speedup 1.0000x reference)
"""Multi-head attention (QKV proj + softmax(QK^T)V) on 8 TRN2 NeuronCores.

Sharding: 8 cores = 4 batches x 2 head-groups (6 heads each). Pure data
parallel - no collectives. Host pre-transposes shards so every on-device
matmul streams with zero on-chip transposes.

The kernel is exp-bound: softmax exp runs only on ScalarE at 1 col/cycle
(~996ns per [128,1024] tile, 192 tiles = 191us hard floor). Everything is
scheduled around keeping ScalarE 100% busy from ~16us on:
  - inputs stream in on TWO hwdge queues (sync + scalar) in deadline order;
    q/k/v are host-packed [128, nch, kt, 512] so one DMA = 128 x 6KB rows
  - per-core pipeline (transposed layouts, d-on-partitions):
      wqT = WqT.T @ qT + bq   [384,2048] (pair p -> head 2p rows 0:64, 2p+1 rows 64:128)
      wv  = vT.T @ WvT + bv   (stored per seq-tile, ones column rides col 64)
      S^T = wkT.T @ wqT per head pair via PE quadrants (tile_position)
      exp on ScalarE (no max subtraction; scores <~70, fp32-safe)
      [out.T; rowsum] = [wv | 1].T @ P^T  (denominator rides the AV matmul)
  - projection units are split into 256-col halves and inserted AFTER each
    iteration's S/exp emission on a precomputed deadline schedule, so a
    DMA-late insert can never starve the exp stream
  - normalize: recip on DVE via [128,8] bounce (partition-serial otherwise),
    partition-broadcast via K=1 fp32 matmul on the idle PE, chain split per
    head to shorten the serial tail.
"""

import sys

if "/opt/trn_rl_repo" not in sys.path:
    sys.path.insert(0, "/opt/trn_rl_repo")

import numpy as np

_TILEPOS = True

_BS, _SEQ, _DM = 4, 2048, 768
_NH, _DH = 12, 64
_GSZ = _DM // 2  # 384 dims per head-group
_NCORES = 8

_KT = _DM // 128   # 6 contraction tiles
_ST = _SEQ // 128  # 16 seq tiles (key positions)
_QC = _SEQ // 512  # 4 query chunks
_NP = _GSZ // 128  # 3 head pairs

_compiled = None


def _insert_schedule():
    """global-iter -> list of insert items. iter = 64*p + 16*qch + kt.

    Items: ("V", st) one V-projection unit (6 matmuls of 384 cols);
           ("qk", which, m, nch, c0, w) partial q/k unit (6 matmuls of w cols).
    Emitted AFTER the iteration's S/exp + AV, so a DMA-late insert never
    delays the exp already issued for this iteration.
    """
    ins = {}

    def add(it, item):
        ins.setdefault(it, []).append(item)

    # V units: wv[st] consumed by AV at iter st of pair-0 qchunk 0. Placed
    # 2 iters early (except V0/V1 at iter 0) so the DVE bias-add overlaps
    # PE work instead of stalling the in-order PE right before AV(st).
    # (A just-in-time 1/iter spread measured slower: the per-iteration
    # bias-add wait before each AV costs more than the clump at iter 0.)
    add(0, ("V", 0, 0))
    add(0, ("V", 1, 0))
    for st in range(2, _ST):
        add(st - 2, ("V", st, 0))
    # pair-0 k/q: k(0,nch) needed before S kt=4nch (issued 2 early);
    # q(0,qch) before iter 16qch. Full 512-col units: ldweights fully
    # hidden under the 512-col streams (256-col halves are barely
    # stream-bound), and half the insert count means less scheduling
    # friction. Ramp deadlines still hold at these placements.
    add(1, ("qk", "k", 0, 1, 0, 512))
    add(5, ("qk", "k", 0, 2, 0, 512))
    add(9, ("qk", "k", 0, 3, 0, 512))
    add(12, ("qk", "q", 0, 1, 0, 512))
    add(26, ("qk", "q", 0, 2, 0, 512))
    add(42, ("qk", "q", 0, 3, 0, 512))
    # pair 1 (deadlines: k(1,n) by 62+4n, q(1,c) by 62+16c)
    for nch in range(4):
        add(46 + 3 * nch, ("qk", "k", 1, nch, 0, 512))
    add(59, ("qk", "q", 1, 0, 0, 512))
    for qch, base in ((1, 74), (2, 90), (3, 105)):
        add(base, ("qk", "q", 1, qch, 0, 512))
    # pair 2 (deadlines: k(2,n) by 126+4n, q(2,c) by 126+16c)
    for nch in range(4):
        add(110 + 3 * nch, ("qk", "k", 2, nch, 0, 512))
    add(122, ("qk", "q", 2, 0, 0, 512))
    for qch, base in ((1, 138), (2, 154), (3, 170)):
        add(base, ("qk", "q", 2, qch, 0, 512))

    # safety: every unit placed before its consumer
    for it, items in ins.items():
        for item in items:
            if item[0] in ("V", "V2"):
                assert it <= 64 * item[2] + item[1], (it, item)
            else:
                _, which, m, nch, c0, w = item
                if which == "k":
                    dl = 64 * m + 4 * nch + 2 * (c0 // 256) - 2
                else:
                    dl = 64 * m + 16 * nch - 2
                assert it <= max(dl, 1) or m == 0 and nch <= 1, (it, item)
    return ins


def _build():
    import concourse.bass as bass  # noqa: F401
    import concourse.mybir as mybir
    import concourse.tile as tile
    from concourse import bacc

    f32 = mybir.dt.float32
    bf16 = mybir.dt.bfloat16
    AF = mybir.ActivationFunctionType

    nc = bacc.Bacc("TRN2", target_bir_lowering=False, debug=False)

    # q/k/v packed [p, nch, kt, c]: one nch slice = 128 descriptors x 6KB
    q_pk = nc.dram_tensor("q_pk", [128, 4, _KT, 512], bf16, kind="ExternalInput")
    k_pk = nc.dram_tensor("k_pk", [128, 4, _KT, 512], bf16, kind="ExternalInput")
    v_pk = nc.dram_tensor("v_pk", [128, 4, _KT, 512], bf16, kind="ExternalInput")
    # Wq/Wk packed [p, m, kt, c]: one m slab = 128 x 1.5KB
    Wq_pk = nc.dram_tensor("Wq_pk", [128, _NP, _KT, 128], bf16, kind="ExternalInput")
    Wk_pk = nc.dram_tensor("Wk_pk", [128, _NP, _KT, 128], bf16, kind="ExternalInput")
    Wv_pk = nc.dram_tensor("Wv_pk", [128, _KT, _GSZ], bf16, kind="ExternalInput")
    # biases packed together: cols 0:3 bqT, 3:6 bkT, 6:390 bv (row-replicated)
    b_all = nc.dram_tensor("b_all", [128, 6 + _GSZ], f32, kind="ExternalInput")
    outT = nc.dram_tensor("outT", [_GSZ, _SEQ], f32, kind="ExternalOutput")

    INSERTS = _insert_schedule()

    with tile.TileContext(nc) as tc:
        with (
            tc.tile_pool(name="persist", bufs=1) as persist,
            tc.tile_pool(name="qkv", bufs=1) as qkv_pool,
            tc.tile_pool(name="w", bufs=1) as w_pool,
            tc.tile_pool(name="psum", bufs=2, space="PSUM") as psum,
            tc.tile_pool(name="att", bufs=4) as att_pool,
        ):
            # ---- persistent SBUF ----
            wqT_sb = [persist.tile([128, _SEQ], bf16, tag=f"wqT{p}", name=f"wqT{p}")
                      for p in range(_NP)]
            wkT_sb = [persist.tile([128, _SEQ], bf16, tag=f"wkT{p}", name=f"wkT{p}")
                      for p in range(_NP)]
            # per seq-tile, per head: [64 wv dims | ones | pad]
            wv_sb = persist.tile([128, _ST, 6, 66], bf16, tag="wv")
            for st in range(_ST):
                nc.vector.memset(wv_sb[:, st, :, 64:65], 1.0)
            ones_sb = persist.tile([1, 64], f32, tag="ones")
            nc.vector.memset(ones_sb[:, :], 1.0)
            ones16 = persist.tile([1, 64], bf16, tag="ones16")
            nc.vector.memset(ones16[:, :], 1.0)

            q_all = qkv_pool.tile([128, 4, _KT, 512], bf16, tag="qa", name="q_all")
            k_all = qkv_pool.tile([128, 4, _KT, 512], bf16, tag="ka", name="k_all")
            v_all = qkv_pool.tile([128, 4, _KT, 512], bf16, tag="va", name="v_all")
            wq_all = w_pool.tile([128, _NP, _KT, 128], bf16, tag="wqa", name="wq_all")
            wk_all = w_pool.tile([128, _NP, _KT, 128], bf16, tag="wka", name="wk_all")
            wv_all = w_pool.tile([128, _KT, _GSZ], bf16, tag="wva", name="wv_all")
            b_sb = persist.tile([128, 6 + _GSZ], f32, tag="b")

            # ---- input DMAs, two hwdge queues, deadline priority order ----
            # Measured: the sync queue sustains ~300KB/us, the scalar queue
            # ~190KB/us; both run concurrently. Scalar queue carries only the
            # exp#1-critical chunks + the later k chunks; sync carries the
            # rest in deadline order.
            nc.scalar.dma_start(k_all[:, 0], k_pk[:, 0])
            nc.scalar.dma_start(q_all[:, 0], q_pk[:, 0])
            nc.scalar.dma_start(k_all[:, 2], k_pk[:, 2])
            nc.scalar.dma_start(k_all[:, 3], k_pk[:, 3])
            nc.scalar.dma_start(q_all[:, 1], q_pk[:, 1])
            nc.scalar.dma_start(q_all[:, 2], q_pk[:, 2])
            nc.scalar.dma_start(q_all[:, 3], q_pk[:, 3])
            nc.scalar.dma_start(wk_all[:, 1], Wk_pk[:, 1])
            nc.scalar.dma_start(wq_all[:, 1], Wq_pk[:, 1])
            nc.scalar.dma_start(wk_all[:, 2], Wk_pk[:, 2])
            nc.scalar.dma_start(wq_all[:, 2], Wq_pk[:, 2])
            nc.sync.dma_start(b_sb[:, :], b_all[:, :])
            nc.sync.dma_start(wk_all[:, 0], Wk_pk[:, 0])
            nc.sync.dma_start(wq_all[:, 0], Wq_pk[:, 0])
            for g in range(2):
                psl = slice(g * 64, (g + 1) * 64)
                nc.sync.dma_start(wv_all[psl], Wv_pk[psl])
            nc.sync.dma_start(v_all[:, 0], v_pk[:, 0])
            nc.sync.dma_start(k_all[:, 1], k_pk[:, 1])
            nc.sync.dma_start(v_all[:, 1], v_pk[:, 1])
            nc.sync.dma_start(v_all[:, 2], v_pk[:, 2])
            nc.sync.dma_start(v_all[:, 3], v_pk[:, 3])

            # ---- projection unit emitters ----
            def emit_v_unit(st, pr, npair=1):
                nch, cc = st // 4, (st % 4) * 128
                d0, dn = 128 * pr, 128 * npair
                nh = 2 * npair
                psv = psum.tile([128, dn], f32, tag="ins", name="psv",
                                padded_shape=[128, 512], bufs=2)
                for t in range(_KT):
                    nc.tensor.matmul(
                        psv[:, :], v_all[:, nch, t, cc:cc + 128],
                        wv_all[:, t, d0:d0 + dn],
                        start=(t == 0), stop=(t == _KT - 1),
                    )
                nc.vector.tensor_add(
                    wv_sb[:, st, 2 * pr:2 * pr + nh, 0:64],
                    psv[:, :].rearrange("p (h d) -> p h d", h=nh),
                    b_sb[:, 6 + d0:6 + d0 + dn].rearrange(
                        "p (h d) -> p h d", h=nh),
                )

            def emit_qk_part(which, m, nch, c0, w):
                ps = psum.tile([128, w], f32, tag="ins", name="psqk",
                               padded_shape=[128, 512], bufs=2)
                x_all, w_all, dst, boff = (
                    (q_all, wq_all, wqT_sb, 0) if which == "q"
                    else (k_all, wk_all, wkT_sb, 3)
                )
                for t in range(_KT):
                    nc.tensor.matmul(
                        ps[:, :], w_all[:, m, t, :],
                        x_all[:, nch, t, c0:c0 + w],
                        start=(t == 0), stop=(t == _KT - 1),
                    )
                nc.vector.tensor_scalar_add(
                    dst[m][:, nch * 512 + c0:nch * 512 + c0 + w],
                    ps[:, :], b_sb[:, boff + m:boff + m + 1])

            def emit_insert(item):
                if item[0] == "V":
                    emit_v_unit(item[1], item[2], npair=3)
                else:
                    _, which, m, nch, c0, w = item
                    emit_qk_part(which, m, nch, c0, w)

            # ---- head: pair-0 qchunk-0 q/k projections + first two V units ----
            # SPINE priority offset: the attention spine (S-pairs, exps, AVs,
            # and the head k00/q00 that gate exp#1) is emitted with a large
            # priority offset so the Tile list-scheduler never displaces
            # ready spine work with projection fillers whose DMA data may be
            # late on real hardware.
            with tc.high_priority():
                emit_qk_part("k", 0, 0, 0, 512)
                emit_qk_part("q", 0, 0, 0, 512)

            # ---- attention stream ----
            # deferred normalize state: (av_sb, recip, hA, hB, qsl) of the
            # previous qchunk; its PE broadcast + muls + out DMAs are emitted
            # a few iterations INTO the next qchunk so the in-order PE stream
            # never stalls on the reciprocal round-trip at a boundary.
            pending = []

            def flush_pending(final=False):
                av_sb, recip, fA, fB, fqsl = pending.pop()
                o_sb = att_pool.tile([64, 1024], f32, tag="o", name="o_sb",
                                     bufs=2)
                if not final:
                    # mid-stream: partition-broadcast on GpSimd (idle engine,
                    # zero PE/PSUM cost; latency hidden by the deferral)
                    bc_sb = att_pool.tile([64, 1024], f32, tag="bc",
                                          name="bc_sb", bufs=2)
                    nc.gpsimd.partition_broadcast(bc_sb[0:64, :],
                                                  recip[0:1, :])
                for h, c0 in ((fA, 0), (fB, 512)):
                    csl = slice(c0, c0 + 512)
                    if final:
                        # final tail: K=1 fp32 matmul on the now-idle PE is
                        # faster than waking GpSimd (~3us with its drain)
                        bc_ps = psum.tile([64, 512], f32, tag="ins",
                                          name="bc_ps",
                                          padded_shape=[128, 512], bufs=2)
                        nc.tensor.matmul(bc_ps[0:64, :], ones16[0:1, 0:64],
                                         recip[0:1, csl],
                                         start=True, stop=True)
                        bc_view = bc_ps[0:64, :]
                    else:
                        bc_view = bc_sb[0:64, csl]
                    nc.vector.tensor_mul(
                        o_sb[0:64, csl], av_sb[0:64, csl], bc_view)
                    nc.sync.dma_start(
                        outT[h * 64:h * 64 + 64, fqsl], o_sb[0:64, csl])

            # Flat 192-iteration stream: the S/exp pipeline runs a constant
            # 2 iterations ahead ACROSS qchunk and pair boundaries, so the
            # exp stream never rebuilds its lookahead at a boundary. Only the
            # AV/normalize stream (which tolerates lag) sees qchunk edges.
            # p tiles are a manual ring (not a pool): the ring recycle is a
            # plain WAR dep that is implied by each ACT's own S-matmul wait,
            # so no standalone pool-semaphore instruction burns ScalarE time.
            p_ring = [persist.tile([128, 1024], bf16, tag=f"pr{j}",
                                   name=f"pr{j}") for j in range(24)]
            p_tiles = {}

            def emit_s_exp(gi):
                if gi < 16:
                    with tc.high_priority():
                        _emit_s_exp(gi)
                    return
                _emit_s_exp(gi)

            def _emit_s_exp(gi):
                sp, r = divmod(gi, 64)
                sqch, skt = divmod(r, 16)
                qsl = slice(sqch * 512, (sqch + 1) * 512)
                ksl = slice(skt * 128, (skt + 1) * 128)
                s_AB = psum.tile([128, 1024], f32, tag="s",
                                 name="sAB", bufs=2)
                nc.tensor.matmul(
                    s_AB[:, 0:512],
                    wkT_sb[sp][0:64, ksl], wqT_sb[sp][0:64, qsl],
                    start=True, stop=True,
                    tile_position=(0, 0) if _TILEPOS else None,
                )
                nc.tensor.matmul(
                    s_AB[:, 512:1024],
                    wkT_sb[sp][64:128, ksl], wqT_sb[sp][64:128, qsl],
                    start=True, stop=True,
                    tile_position=(64, 0) if _TILEPOS else None,
                )
                p_AB = p_ring[gi % 24]
                nc.scalar.activation(p_AB[:, :], s_AB[:, :], AF.Exp)
                p_tiles[gi] = p_AB

            avA = avB = None
            av_defer = []
            emit_s_exp(0)
            emit_s_exp(1)
            for gi in range(_NP * _QC * _ST):
                if gi + 2 < _NP * _QC * _ST:
                    emit_s_exp(gi + 2)
                p, r = divmod(gi, 64)
                qch, kt = divmod(r, 16)
                hA, hB = 2 * p, 2 * p + 1
                if kt == 0:
                    # two 1-bank accumulators: the eviction copy of half A
                    # can overlap AV15 of half B, and next qchunk's AV0.A
                    # only waits on copyA - shrinks the boundary stall
                    avA = psum.tile([65, 512], f32, tag="avA", name="avA",
                                    padded_shape=[128, 512], bufs=1)
                    avB = psum.tile([65, 512], f32, tag="avB", name="avB",
                                    padded_shape=[128, 512], bufs=1)
                if kt == 4 and pending:
                    flush_pending()
                for item in INSERTS.get(gi, ()):
                    emit_insert(item)
                pv = p_tiles.pop(gi)
                # defer the first two AV pairs of each qchunk by two
                # iterations: the boundary S-pairs then run ahead of the
                # accumulator-eviction wait instead of queuing behind it
                av_defer.append((kt, pv))
                if kt < 2:
                    continue
                for dkt, dpv in av_defer:
                    nc.tensor.matmul(
                        avA[0:65, :], wv_sb[:, dkt, hA, 0:65],
                        dpv[:, 0:512],
                        start=(dkt == 0), stop=(dkt == _ST - 1),
                    )
                    nc.tensor.matmul(
                        avB[0:65, :], wv_sb[:, dkt, hB, 0:65],
                        dpv[:, 512:1024],
                        start=(dkt == 0), stop=(dkt == _ST - 1),
                    )
                av_defer.clear()
                if kt != _ST - 1:
                    continue

                # ---- qchunk done: evict + reciprocal (latency-tolerant) ----
                qsl = slice(qch * 512, (qch + 1) * 512)
                av_sb = att_pool.tile([65, 1024], f32, tag="av_sb",
                                      name="av_sb", bufs=2)
                last = gi == _NP * _QC * _ST - 1
                with tc.high_priority():
                    if last:
                        # final tail: sums rows only; the big value copies
                        # are emitted AFTER the reciprocal bounce below so
                        # the DVE runs the recips as soon as the bounce DMA
                        # lands instead of behind the copies
                        nc.vector.tensor_copy(av_sb[64:65, 512:1024],
                                              avB[64:65, :])
                        nc.vector.tensor_copy(av_sb[64:65, 0:512],
                                              avA[64:65, :])
                    else:
                        # mid-stream: A first - next qchunk's AV0.A is the
                        # head of the PE stream and waits only on copyA
                        nc.vector.tensor_copy(av_sb[0:65, 0:512],
                                              avA[0:65, :])
                        nc.vector.tensor_copy(av_sb[0:65, 512:1024],
                                              avB[0:65, :])
                # reciprocal is partition-serial on DVE; bounce the sums
                # through [128,4] so all lanes work, then bounce back.
                rp = att_pool.tile([128, 8], f32, tag="rp", name="rp",
                                   bufs=2)
                if last:
                    # bf16 recip: the final-flush broadcast matmul runs
                    # single-pass bf16 instead of two-pass fp32 (~1.5us off
                    # the serial tail; ~0.4% on the denominator is inside
                    # the accuracy budget)
                    rp2 = att_pool.tile([128, 8], bf16, tag="rp2f",
                                        name="rp2f", bufs=1)
                    recip = att_pool.tile([1, 1024], bf16, tag="recipf",
                                          name="recipf", bufs=1)
                else:
                    rp2 = att_pool.tile([128, 8], f32, tag="rp2", name="rp2",
                                        bufs=2)
                    recip = att_pool.tile([1, 1024], f32, tag="recip",
                                          name="recip", bufs=2)
                for c0 in (0, 512):
                    csl = slice(c0, c0 + 512)
                    rsl = slice(c0 // 128, c0 // 128 + 4)
                    nc.sync.dma_start(rp[0:128, rsl], av_sb[64:65, csl])
                    with nc.allow_low_precision(reason="bf16 recip bcast"):
                        nc.vector.reciprocal(rp2[0:128, rsl], rp[0:128, rsl])
                    nc.sync.dma_start(recip[0:1, csl], rp2[0:128, rsl])
                if last:
                    # deferred big value copies (only the muls need them)
                    nc.vector.tensor_copy(av_sb[0:64, 512:1024], avB[0:64, :])
                    nc.vector.tensor_copy(av_sb[0:64, 0:512], avA[0:64, :])
                pending.append((av_sb, recip, hA, hB, qsl))

            flush_pending(final=True)

    nc.compile()
    return nc


def _get_compiled():
    global _compiled
    if _compiled is None:
        _compiled = _build()
    return _compiled


def make_in_maps(q, k, v, Wq, bq, Wk, bk, Wv, bv):
    import ml_dtypes

    bf16 = ml_dtypes.bfloat16

    def pack_x(xT):  # [768, 2048] -> [128, 4nch, 6kt, 512]
        return np.ascontiguousarray(
            xT.reshape(_KT, 128, 4, 512).transpose(1, 2, 0, 3)).astype(bf16)

    def pack_w(WT):  # [768, 384] -> [128, 3m, 6kt, 128]
        return np.ascontiguousarray(
            WT.reshape(_KT, 128, _NP, 128).transpose(1, 2, 0, 3)).astype(bf16)

    in_maps = []
    for c in range(_NCORES):
        b, g = c // 2, c % 2
        gsl = slice(g * _GSZ, (g + 1) * _GSZ)
        b_pack = np.concatenate([
            np.asarray(bq)[gsl].reshape(3, 128).T,
            np.asarray(bk)[gsl].reshape(3, 128).T,
            np.tile(np.asarray(bv)[gsl][None, :], (128, 1)),
        ], axis=1).astype(np.float32)
        in_maps.append({
            "q_pk": pack_x(np.asarray(q)[b].T),
            "k_pk": pack_x(np.asarray(k)[b].T),
            "v_pk": pack_x(np.asarray(v)[b].T),
            "Wq_pk": pack_w(np.asarray(Wq)[gsl, :].T),
            "Wk_pk": pack_w(np.asarray(Wk)[gsl, :].T),
            "Wv_pk": np.ascontiguousarray(np.asarray(Wv)[gsl, :].T.reshape(
                _KT, 128, _GSZ).transpose(1, 0, 2)).astype(bf16),
            "b_all": np.ascontiguousarray(b_pack),
        })
    return in_maps


def assemble_out(results):
    out = np.zeros((_BS, _SEQ, _DM), np.float32)
    for c in range(_NCORES):
        b, g = c // 2, c % 2
        out[b, :, g * _GSZ:(g + 1) * _GSZ] = np.asarray(
            results[c]["outT"], np.float32
        ).T
    return out


def kernel(q, k, v, Wq, bq, Wk, bk, Wv, bv):
    from concourse.bass_utils import run_bass_kernel_spmd

    nc = _get_compiled()
    in_maps = make_in_maps(q, k, v, Wq, bq, Wk, bk, Wv, bv)
    res = run_bass_kernel_spmd(nc, in_maps, core_ids=list(range(_NCORES)))
    return assemble_out(res.results)

